# revision 1
# baseline (speedup 1.0000x reference)
"""GAT-style graph encoder on 8 trn2 NeuronCores.

Reference computation (per exercise row i over kc nodes j):
    kc_Wh = kc_h @ W1; ex_Wh = ex_h @ W1
    e[i,j] = leaky_relu(ex_Wh[i]@a1 + kc_Wh[j]@a2, 0.2)
    att = softmax(where(adj>0, e, -9e15), axis=1)
    new_kc = att @ kc_Wh; ex_Eh = ex_h @ E
    out = elu(concat([new_kc, new_kc*ex_Eh]) @ rd_w.T + rd_b)

Strategy: row-shard exercises over 8 cores (1250 rows each, padded to 1280).
On-chip everything lives in a transposed [kc_or_feature, exercise] layout so
softmax numerator/denominator are plain PE matmuls contracting over the kc
partition axis -- no on-chip transposes.  Masking is a multiply (adj is 0/1)
on the exp'd logits; since logits are bounded (|e| <~ 15) the softmax is
computed without max-subtraction, exactly matching reference semantics to
f32 roundoff.  ex_a1 enters via the per-partition broadcast tile, kc_a2 via
the activation bias port, both folded through W1 on the host (weight-only
algebra: ex_Wh@a1 == ex_h@(W1@a1)).
"""

import ml_dtypes
import numpy as np

import concourse.bacc as bacc
import concourse.bass as bass
import concourse.mybir as mybir
from concourse.alu_op_type import AluOpType
from concourse.bass_utils import run_bass_kernel_spmd
from concourse.tile import TileContext

F32 = mybir.dt.float32
F32R = mybir.dt.float32r
BF16 = mybir.dt.bfloat16
AF = mybir.ActivationFunctionType

P = 128
D = 256                    # feature dim
NKC = 2048                 # padded kc count (2000 real)
KCH = NKC // P             # 16 kc chunks
M = 1280                   # padded exercise rows per core (1250 real)
MBS = (512, 512, 256)      # m blocks (>=256 keeps float32r at 1 cyc/row)
MOFF = (0, 512, 1024)
NCORES = 8
ROWS = 1250
N_E = 10000
ALPHA = 0.2
# A: 0/1 multiply-mask (ACT leaky+exp, DVE mask)
# B: fold, Pool tt, ACT leaky | C: fold, DVE tt, ACT leaky
# D: fold, Pool tt, DVE leaky | E: fold, DVE tt, DVE leaky
VARIANTS = ("B", "E", "A", "D", "B", "C", "A", "D")


def _build():
    nc = bacc.Bacc("TRN2", target_bir_lowering=False, debug=False,
                   num_devices=NCORES)
    exT = nc.declare_dram_parameter("exT", [2 * P, M], F32R, isOutput=False)
    adjT = nc.declare_dram_parameter("adjT", [NKC, M], BF16, isOutput=False)
    kcT = nc.declare_dram_parameter("kcT", [2 * P, NKC], F32R, isOutput=False)
    W1e = nc.declare_dram_parameter("W1e", [2 * P, D + 2], F32R, isOutput=False)
    w1a1 = nc.declare_dram_parameter("w1a1", [2 * P, 1], F32R, isOutput=False)
    Em = nc.declare_dram_parameter("Em", [2 * P, D], F32R, isOutput=False)
    rdwT = nc.declare_dram_parameter("rdwT", [4 * P, D], F32R, isOutput=False)
    rdb = nc.declare_dram_parameter("rdb", [2 * P, 1], F32, isOutput=False)
    outT = nc.declare_dram_parameter("outT", [2 * P, M], F32, isOutput=True)

    with TileContext(nc) as tc:
        with tc.tile_pool(name="const", bufs=1) as cpool, \
             tc.tile_pool(name="acc_ps", bufs=1, space="PSUM") as apool, \
             tc.tile_pool(name="out_ps", bufs=1, space="PSUM") as opool, \
             tc.tile_pool(name="mwork", bufs=8) as mpool, \
             tc.tile_pool(name="post", bufs=2) as qpool:
            kcT_sb, W1e_sb, Em_sb, w1a1_sb, rdb_sb, exT_sb = [], [], [], [], [], []
            for c in range(2):
                t = cpool.tile([P, NKC], F32R, tag=f"kcT{c}")
                nc.sync.dma_start(out=t[:], in_=kcT[c * P:(c + 1) * P, :])
                kcT_sb.append(t)
                t = cpool.tile([P, D + 2], F32R, tag=f"W1e{c}")
                nc.sync.dma_start(out=t[:], in_=W1e[c * P:(c + 1) * P, :])
                W1e_sb.append(t)
                t = cpool.tile([P, D], F32R, tag=f"Em{c}")
                nc.sync.dma_start(out=t[:], in_=Em[c * P:(c + 1) * P, :])
                Em_sb.append(t)
                t = cpool.tile([P, 1], F32R, tag=f"w1a1{c}")
                nc.sync.dma_start(out=t[:], in_=w1a1[c * P:(c + 1) * P, :])
                w1a1_sb.append(t)
                t = cpool.tile([P, 1], F32, tag=f"rdb{c}")
                nc.sync.dma_start(out=t[:], in_=rdb[c * P:(c + 1) * P, :])
                rdb_sb.append(t)
                t = cpool.tile([P, M], F32R, tag=f"exT{c}")
                nc.sync.dma_start(out=t[:], in_=exT[c * P:(c + 1) * P, :])
                exT_sb.append(t)
            rdwT_sb = []
            for dd in range(4):
                t = cpool.tile([P, D], F32R, tag=f"rdwT{dd}")
                nc.sync.dma_start(out=t[:], in_=rdwT[dd * P:(dd + 1) * P, :])
                rdwT_sb.append(t)
            ones1f = cpool.tile([1, P], F32, tag="ones1f")
            nc.vector.memset(ones1f[:], 1.0)
            ones1 = cpool.tile([1, P], F32R, tag="ones1")
            nc.scalar.copy(ones1[:], ones1f[:])
            ones128f = cpool.tile([P, 1], F32, tag="ones128f")
            nc.vector.memset(ones128f[:], 1.0)
            ones128 = cpool.tile([P, 1], F32R, tag="ones128")
            nc.scalar.copy(ones128[:], ones128f[:])

            # ---- setup (emitted in dependency-criticality order:
            # exa1b gates every main-loop block, kcWh[kk] gates chunk kk,
            # exEhT is needed only at the post stage of block 0)
            kcWh, kca2 = [], []
            exa1b = cpool.tile([P, M], F32, tag="exa1b")
            exa1_sb = cpool.tile([1, M], F32R, tag="exa1_sb")
            exEhT = [cpool.tile([P, M], F32, tag=f"exEhT{d}", name=f"exEhT{d}")
                     for d in range(2)]
            with tc.tile_pool(name="setup_ps", bufs=2, space="PSUM") as spool:
                for b in range(3):
                    ms = slice(MOFF[b], MOFF[b] + MBS[b])
                    ps = spool.tile([1, MBS[b]], F32, tag="misc_ps",
                                    name=f"row_ps{b}")
                    for c in range(2):
                        nc.tensor.matmul(ps[:], w1a1_sb[c][:],
                                         exT_sb[c][:, ms],
                                         start=(c == 0), stop=(c == 1))
                    nc.vector.tensor_copy(exa1_sb[:, ms], ps[:])
                    psb = spool.tile([P, MBS[b]], F32, tag="misc_ps",
                                     name=f"bc_ps{b}")
                    nc.tensor.matmul(psb[:], ones1[:], exa1_sb[:, ms],
                                     start=True, stop=True)
                    nc.vector.tensor_copy(exa1b[:, ms], psb[:])
                for kk in range(KCH):
                    ps = spool.tile([P, D + 2], F32, tag="kcwh_ps")
                    for c in range(2):
                        nc.tensor.matmul(
                            ps[:], kcT_sb[c][:, kk * P:(kk + 1) * P],
                            W1e_sb[c][:], start=(c == 0), stop=(c == 1))
                    t = cpool.tile([P, D], F32R, tag=f"kcWh{kk}",
                                   name=f"kcWh{kk}")
                    if kk % 2 == 0:
                        nc.scalar.copy(t[:], ps[:, 0:D])
                    else:
                        nc.vector.tensor_copy(t[:], ps[:, 0:D])
                    kcWh.append(t)
                    tb = cpool.tile([P, 1], F32, tag=f"kca2_{kk}",
                                    name=f"kca2_{kk}")
                    nc.scalar.copy(tb[:], ps[:, D:D + 1])
                    kca2.append(tb)
                for d in range(2):
                    for b in range(3):
                        ms = slice(MOFF[b], MOFF[b] + MBS[b])
                        pse = spool.tile([P, MBS[b]], F32, tag="misc_ps",
                                         name=f"eh_ps{b}_{d}")
                        for c in range(2):
                            nc.tensor.matmul(
                                pse[:], Em_sb[c][:, d * P:(d + 1) * P],
                                exT_sb[c][:, ms], start=(c == 0), stop=(c == 1))
                        nc.scalar.copy(exEhT[d][:, ms], pse[:])

            # ---- main: masked softmax attention + aggregation + readout.
            # adjT row encoding is per-chunk (host-matched): chunks with
            # kk % 4 == 2 carry adj as 0/1 (multiply mask); all others carry
            # 100*(adj-1), i.e. 0 / -100, folded into the logits so that
            # leaky(-100+s) -> exp ~ 2e-9 ~ 0.
            for b in range(3):
                mb = MBS[b]
                ms = slice(MOFF[b], MOFF[b] + mb)
                n0 = apool.tile([P, mb], F32, tag="n0")
                n1 = apool.tile([P, mb], F32, tag="n1")
                sS = apool.tile([1, mb], F32, tag="sS")
                for kk in range(KCH):
                    adjf = mpool.tile([P, mb], BF16, tag="adjf", bufs=12)
                    nc.sync.dma_start(out=adjf[:],
                                      in_=adjT[kk * P:(kk + 1) * P, ms])
                    # 8-chunk rotation balancing ACT/DVE/Pool; see VARIANTS
                    v = VARIANTS[kk % 8]
                    ptm = mpool.tile([P, mb], F32R, tag="ptm")
                    if v == "A":  # multiply-mask: leaky+exp ACT, mask DVE
                        et = mpool.tile([P, mb], F32, tag="et")
                        nc.scalar.activation(et[:], exa1b[:, ms], AF.Prelu,
                                             bias=kca2[kk][:], alpha=ALPHA)
                        pt = mpool.tile([P, mb], F32, tag="pt")
                        nc.scalar.activation(pt[:], et[:], AF.Exp)
                        nc.vector.tensor_mul(ptm[:], pt[:], adjf[:])
                    else:         # logit-fold variants
                        tt_eng = nc.gpsimd if v in ("B", "D") else nc.vector
                        tmp = mpool.tile([P, mb], F32, tag="tmp")
                        tt_eng.tensor_add(tmp[:], adjf[:], exa1b[:, ms])
                        et = mpool.tile([P, mb], F32, tag="et")
                        if v in ("B", "C"):   # leaky on ACT
                            nc.scalar.activation(et[:], tmp[:], AF.Prelu,
                                                 bias=kca2[kk][:], alpha=ALPHA)
                        else:                 # leaky on DVE
                            s02 = mpool.tile([P, mb], F32, tag="s02")
                            nc.vector.tensor_scalar(
                                s02[:], tmp[:], kca2[kk][:], ALPHA,
                                AluOpType.add, AluOpType.mult)
                            nc.vector.scalar_tensor_tensor(
                                et[:], tmp[:], kca2[kk][:], s02[:],
                                AluOpType.add, AluOpType.max)
                        nc.scalar.activation(ptm[:], et[:], AF.Exp)
                    st, sp = (kk == 0), (kk == KCH - 1)
                    nc.tensor.matmul(n0[:], kcWh[kk][:, 0:P], ptm[:],
                                     start=st, stop=sp)
                    nc.tensor.matmul(n1[:], kcWh[kk][:, P:2 * P], ptm[:],
                                     start=st, stop=sp)
                    nc.tensor.matmul(sS[:], ones128[:], ptm[:],
                                     start=st, stop=sp)
                srow = qpool.tile([1, mb], F32R, tag="srow")
                with nc.allow_low_precision(reason="f32r storage is full f32"):
                    nc.vector.reciprocal(srow[:], sS[:])
                sbps = opool.tile([P, mb], F32, tag="u")
                nc.tensor.matmul(sbps[:], ones1[:], srow[:],
                                 start=True, stop=True)
                sinvb = qpool.tile([P, mb], F32, tag="sinvb")
                nc.vector.tensor_copy(sinvb[:], sbps[:])
                nk0 = qpool.tile([P, mb], F32R, tag="nk0")
                nc.vector.tensor_mul(nk0[:], n0[:], sinvb[:])
                nk1 = qpool.tile([P, mb], F32R, tag="nk1")
                nc.vector.tensor_mul(nk1[:], n1[:], sinvb[:])
                t0 = qpool.tile([P, mb], F32R, tag="t0")
                nc.gpsimd.tensor_mul(t0[:], nk0[:], exEhT[0][:, ms])
                t1 = qpool.tile([P, mb], F32R, tag="t1")
                nc.gpsimd.tensor_mul(t1[:], nk1[:], exEhT[1][:, ms])
                feat = [nk0, nk1, t0, t1]
                for oo in range(2):
                    ups = opool.tile([P, mb], F32, tag="u")
                    for dd in range(4):
                        nc.tensor.matmul(
                            ups[:], rdwT_sb[dd][:, oo * P:(oo + 1) * P],
                            feat[dd][:], start=(dd == 0), stop=(dd == 3))
                    # elu(x) = max(x,0) + exp(min(x,0)) - 1,  x = ups + rd_b
                    tmin = qpool.tile([P, mb], F32, tag="tmin")
                    nc.vector.tensor_scalar(tmin[:], ups[:], rdb_sb[oo][:],
                                            0.0, AluOpType.add, AluOpType.min)
                    eneg = qpool.tile([P, mb], F32, tag="eneg")
                    nc.scalar.activation(eneg[:], tmin[:], AF.Exp)
                    tmax = qpool.tile([P, mb], F32, tag="tmax")
                    nc.vector.tensor_scalar(tmax[:], ups[:], rdb_sb[oo][:],
                                            0.0, AluOpType.add, AluOpType.max)
                    res = qpool.tile([P, mb], F32, tag="res")
                    nc.vector.scalar_tensor_tensor(res[:], tmax[:], -1.0,
                                                   eneg[:], AluOpType.add,
                                                   AluOpType.add)
                    nc.sync.dma_start(out=outT[oo * P:(oo + 1) * P, ms],
                                      in_=res[:])
    nc.finalize()
    return nc


_PROGRAM = None


def _get_program():
    global _PROGRAM
    if _PROGRAM is None:
        _PROGRAM = _build()
    return _PROGRAM


def _in_maps(exercise_h, kc_h, adj, W1, E, a, rd_w, rd_b):
    f = np.float32
    a1 = np.ascontiguousarray(a[:D, 0], dtype=f)
    a2 = np.ascontiguousarray(a[D:, 0], dtype=f)
    W1 = np.asarray(W1, dtype=f)
    w1a2 = W1 @ a2
    W1e = np.concatenate([W1, w1a2[:, None],
                          np.zeros((D, 1), f)], axis=1)      # [256, 258]
    w1a1 = (W1 @ a1)[:, None]                                 # [256, 1]
    kcT = np.zeros((2 * P, NKC), dtype=f)
    kcT[:, :2000] = np.asarray(kc_h, dtype=f).T
    Em = np.ascontiguousarray(np.asarray(E, dtype=f))
    rdwT = np.ascontiguousarray(np.asarray(rd_w, dtype=f).T)  # [512, 256]
    rdb = np.asarray(rd_b, dtype=f)[:, None]                  # [256, 1]
    shared = {"kcT": kcT, "W1e": np.ascontiguousarray(W1e),
              "w1a1": np.ascontiguousarray(w1a1), "Em": Em,
              "rdwT": rdwT, "rdb": np.ascontiguousarray(rdb)}
    maps = []
    for c in range(NCORES):
        sl = slice(c * ROWS, (c + 1) * ROWS)
        exT_c = np.zeros((2 * P, M), dtype=f)
        exT_c[:, :ROWS] = np.asarray(exercise_h[sl], dtype=f).T
        adjx = np.asarray(adj[sl], dtype=np.float32).T  # [2000, 1250] of 0/1
        adjT_c = np.zeros((NKC, M), dtype=ml_dtypes.bfloat16)
        for kk in range(KCH):
            rs = slice(kk * P, (kk + 1) * P)
            blk = np.zeros((P, M), dtype=np.float32)
            nreal = max(0, min(2000 - kk * P, P))
            if VARIANTS[kk % 8] == "A":   # multiply-mask chunk: 0/1
                blk[:nreal, :ROWS] = adjx[kk * P:kk * P + nreal]
                blk[:nreal, ROWS:] = 1.0   # pad rows finite
                blk[nreal:, :] = 0.0       # pad kc nodes masked out
            else:                 # logit-fold chunk: 0/-100, pad kc = -100
                blk[:nreal, :ROWS] = (adjx[kk * P:kk * P + nreal] - 1.0) * 100.0
                blk[:nreal, ROWS:] = 0.0
                blk[nreal:, :] = -100.0
            adjT_c[rs] = blk
        del adjx
        maps.append({"exT": exT_c, "adjT": adjT_c, **shared})
    return maps


def kernel(exercise_h, kc_h, adj, W1, E, a, rd_w, rd_b):
    nc = _get_program()
    maps = _in_maps(exercise_h, kc_h, adj, W1, E, a, rd_w, rd_b)
    res = run_bass_kernel_spmd(nc, maps, list(range(NCORES))).results
    out = np.empty((N_E, D), dtype=np.float32)
    for c in range(NCORES):
        out[c * ROWS:(c + 1) * ROWS] = res[c]["outT"][:, :ROWS].T
    return out



# revision 8
# speedup vs baseline: 1.6449x; 1.6449x over previous
"""GAT-style graph encoder on 8 trn2 NeuronCores — v2.

Reference (per exercise i over kc nodes j):
    kc_Wh = kc_h @ W1; ex_Wh = ex_h @ W1
    e[i,j] = leaky_relu(u_i + v_j, 0.2),  u = ex_Wh@a1, v = kc_Wh@a2
    att = softmax(where(adj>0, e, -9e15), axis=1)
    new_kc = att @ kc_Wh; ex_Eh = ex_h @ E
    out = elu(concat([new_kc, new_kc*ex_Eh]) @ rd_w.T + rd_b)

Strategy (row-shard exercises over 8 cores, 1250 rows -> padded 1280):
Because the pre-activation logit is separable (u_i + v_j), exp(leaky(s) - r_i)
with the softmax-shift r_i = u_i + c factors into rank-1 products:
    p[j,i] = max(C'_j * 1, D_j * B'_i),  C' = e^{v-c}, D = e^{0.2 v},
    B' = e^{-0.8 u - c}   (host-computed rows; softmax is invariant to any
    per-row scale, so the shift is exact algebra, not an approximation).
On-chip per kc-chunk the whole attention numerator needs just TWO elementwise
passes: a 4x-mode DVE tensor_scalar  (B'_bcast * D_j) max C'_j  and one
mask multiply with adj in {0,1}.  kc nodes are host-sorted by v so the two
top chunks (which carry ~90% of softmax mass) aggregate in bf16 while the
14 tail chunks use fp8e4 DoubleRow matmuls (2 k-tiles per instr at 0.5
cyc/row).  Readout runs in bf16.  The final per-row softmax division, +rd_b
and elu are applied on the host during unshard (per-row scalar epilogue).
"""

import ml_dtypes
import numpy as np

import concourse.bacc as bacc
import concourse.bass as bass
import concourse.mybir as mybir
from concourse.alu_op_type import AluOpType
from concourse.bass_utils import run_bass_kernel_spmd
from concourse.tile import TileContext

F32 = mybir.dt.float32
BF16 = mybir.dt.bfloat16
FP8 = mybir.dt.float8e4
AF = mybir.ActivationFunctionType
DR = mybir.MatmulPerfMode.DoubleRow

P = 128
D = 256
NKC = 2048
KCH = 16                    # kc chunks
NBF = 2                     # leading (high-v) chunks aggregated in bf16
NPAIR = (KCH - NBF) // 2    # fp8 DoubleRow chunk pairs
M = 1280                    # padded exercise rows per core
MBS = (512, 512, 256)
MOFF = (0, 512, 1024)
NCORES = 8
ROWS = 1250
N_E = 10000
SCALE = 128.0               # fp8 range scale folded into B'/C' (cancels in n/s)
# chunks whose mask multiply runs on DVE (fp8 out, 1x) vs Pool
DVE_MASK = frozenset((3, 6, 9, 12, 15))

NP_BF16 = ml_dtypes.bfloat16
NP_FP8 = ml_dtypes.float8_e4m3


def _build():
    nc = bacc.Bacc("TRN2", target_bir_lowering=False, debug=False,
                   num_devices=NCORES)
    adjg = [nc.declare_dram_parameter(f"adjg{g}", [P, 4 * M], FP8,
                                      isOutput=False) for g in range(4)]
    exTb = nc.declare_dram_parameter("exTb", [P, 2 * M], BF16, isOutput=False)
    kcWhT8 = nc.declare_dram_parameter("kcWhT8", [P, NPAIR * 512], FP8,
                                       isOutput=False)
    kcWhTb = nc.declare_dram_parameter("kcWhTb", [P, NBF * 256], BF16,
                                       isOutput=False)
    EmB = nc.declare_dram_parameter("EmB", [P, 2 * 256], BF16, isOutput=False)
    rdwB = nc.declare_dram_parameter("rdwB", [P, 4 * 256], BF16,
                                     isOutput=False)
    rowB = nc.declare_dram_parameter("rowB", [1, M], BF16, isOutput=False)
    scal = nc.declare_dram_parameter("scal", [P, 32], F32, isOutput=False)
    outB = nc.declare_dram_parameter("outB", [P, 2 * M], BF16, isOutput=True)
    srow = nc.declare_dram_parameter("srow", [1, M], F32, isOutput=True)

    with TileContext(nc) as tc:
        with tc.tile_pool(name="const", bufs=1) as cpool, \
             tc.tile_pool(name="acc_ps", bufs=2, space="PSUM") as apool, \
             tc.tile_pool(name="out_ps", bufs=2, space="PSUM") as opool, \
             tc.tile_pool(name="mwork", bufs=6) as mpool, \
             tc.tile_pool(name="post", bufs=2) as qpool:
            # ---- const loads (priority: adj group 0 gates the first chunks)
            adj_sb = []
            for g in range(4):
                t = cpool.tile([P, 4 * M], FP8, tag=f"adjg{g}", name=f"adjg{g}")
                nc.sync.dma_start(out=t[:], in_=adjg[g][:, :])
                adj_sb.append(t)
            scal_sb = cpool.tile([P, 32], F32, tag="scal")
            nc.sync.dma_start(out=scal_sb[:], in_=scal[:, :])
            rowB_sb = cpool.tile([1, M], BF16, tag="rowB")
            nc.sync.dma_start(out=rowB_sb[:], in_=rowB[:, :])
            kcb_sb = cpool.tile([P, NBF * 256], BF16, tag="kcWhTb")
            nc.sync.dma_start(out=kcb_sb[:], in_=kcWhTb[:, :])
            # 4-D: [p, (pair,target), ktile=2, m] so DoubleRow APs expose Num=2
            kc8_sb = cpool.tile([P, NPAIR * 2, 2, P], FP8, tag="kcWhT8")
            nc.sync.dma_start(out=kc8_sb[:], in_=kcWhT8[:, :])
            exT_sb = cpool.tile([P, 2 * M], BF16, tag="exTb")
            nc.sync.dma_start(out=exT_sb[:], in_=exTb[:, :])
            EmB_sb = cpool.tile([P, 2 * 256], BF16, tag="EmB")
            nc.sync.dma_start(out=EmB_sb[:], in_=EmB[:, :])
            rdw_sb = cpool.tile([P, 4 * 256], BF16, tag="rdwB")
            nc.sync.dma_start(out=rdw_sb[:], in_=rdwB[:, :])

            ones1b = cpool.tile([1, P], BF16, tag="ones1b")
            nc.vector.memset(ones1b[:], 1.0)
            onesb = cpool.tile([P, 1], BF16, tag="onesb")
            nc.vector.memset(onesb[:], 1.0)
            ones8 = cpool.tile([P, 2, 16], FP8, tag="ones8")
            nc.vector.memset(ones8[:], 1.0)

            Bb = cpool.tile([P, M], BF16, tag="Bb")          # B' broadcast
            exEhT = [cpool.tile([P, M], BF16, tag=f"exEhT{d}", name=f"exEhT{d}")
                     for d in (0, 1)]
            outB_sb = cpool.tile([P, 2 * M], BF16, tag="outB_sb")
            srow_sb = cpool.tile([1, M], F32, tag="srow_sb")

            # ---- setup: B' broadcast + ex_Eh (bf16 matmuls, ACT copies)
            # setup psum shares the readout pool (tag "raw"), disjoint in time
            for b in range(3):
                ms = slice(MOFF[b], MOFF[b] + MBS[b])
                ps = opool.tile([P, MBS[b]], F32, tag="raw", name=f"bb_ps{b}")
                nc.tensor.matmul(ps[:], ones1b[:], rowB_sb[:, ms],
                                 start=True, stop=True)
                nc.scalar.copy(Bb[:, ms], ps[:])
            for d in range(2):
                for b in range(3):
                    ms = slice(MOFF[b], MOFF[b] + MBS[b])
                    ps = opool.tile([P, MBS[b]], F32, tag="raw",
                                    name=f"eh_ps{d}_{b}")
                    for c in range(2):
                        nc.tensor.matmul(
                            ps[:],
                            EmB_sb[:, c * 256 + d * P:c * 256 + (d + 1) * P],
                            exT_sb[:, c * M + MOFF[b]:c * M + MOFF[b] + MBS[b]],
                            start=(c == 0), stop=(c == 1))
                    nc.scalar.copy(exEhT[d][:, ms], ps[:])

            # ---- main: per m-block masked-exp + aggregation + readout
            for b in range(3):
                mb = MBS[b]
                ms = slice(MOFF[b], MOFF[b] + mb)
                n0 = apool.tile([P, mb], F32, tag="n0")
                n1 = apool.tile([P, mb], F32, tag="n1")
                sS = apool.tile([1, mb], F32, tag="sS")

                def adjsl(kk):
                    g, o = divmod(kk, 4)
                    return adj_sb[g][:, o * M + MOFF[b]:o * M + MOFF[b] + mb]

                def q_of(kk):
                    q = mpool.tile([P, mb], BF16, tag="q")
                    # (B'_i * D_j) max C'_j : whole unmasked exp in one 4x op
                    nc.vector.tensor_scalar(
                        q[:], Bb[:, ms], scal_sb[:, 16 + kk:17 + kk],
                        scal_sb[:, kk:kk + 1], AluOpType.mult, AluOpType.max)
                    return q

                # bf16 chunks (top softmax mass)
                for kk in range(NBF):
                    q = q_of(kk)
                    ptmb = mpool.tile([P, mb], BF16, tag="ptmb")
                    nc.gpsimd.tensor_mul(ptmb[:], q[:], adjsl(kk))
                    st = (kk == 0)
                    nc.tensor.matmul(n0[:], kcb_sb[:, kk * 256:kk * 256 + P],
                                     ptmb[:], start=st, stop=False)
                    nc.tensor.matmul(n1[:], kcb_sb[:, kk * 256 + P:(kk + 1) * 256],
                                     ptmb[:], start=st, stop=False)
                    nc.tensor.matmul(sS[:], onesb[:], ptmb[:],
                                     start=st, stop=False)
                # fp8 DoubleRow pairs
                for pr in range(NPAIR):
                    ptm8 = mpool.tile([P, 2, mb], FP8, tag="ptm8")
                    for h in range(2):
                        kk = NBF + 2 * pr + h
                        q = q_of(kk)
                        eng = nc.vector if kk in DVE_MASK else nc.gpsimd
                        eng.tensor_mul(ptm8[:, h, :], q[:], adjsl(kk))
                    sp = (pr == NPAIR - 1)
                    nc.tensor.matmul(n0[:], kc8_sb[:, 2 * pr, :, :], ptm8[:],
                                     start=False, stop=sp, perf_mode=DR)
                    nc.tensor.matmul(n1[:], kc8_sb[:, 2 * pr + 1, :, :],
                                     ptm8[:], start=False, stop=sp,
                                     perf_mode=DR)
                    nc.tensor.matmul(sS[:], ones8[:, :, 0:1], ptm8[:],
                                     start=False, stop=sp, perf_mode=DR)

                # ---- post: features, readout, stage out
                nc.vector.tensor_copy(srow_sb[:, ms], sS[:])
                ncf = []
                for t in range(2):
                    nt = qpool.tile([P, mb], BF16, tag=f"nc{t}", name=f"nc{t}")
                    nc.scalar.copy(nt[:], (n0 if t == 0 else n1)[:])
                    ncf.append(nt)
                tf = []
                for t in range(2):
                    tt = qpool.tile([P, mb], BF16, tag=f"t{t}", name=f"tt{t}")
                    nc.vector.tensor_mul(tt[:], ncf[t][:], exEhT[t][:, ms])
                    tf.append(tt)
                feats = [ncf[0], ncf[1], tf[0], tf[1]]
                for oo in range(2):
                    raw = opool.tile([P, mb], F32, tag="raw")
                    for dd in range(4):
                        nc.tensor.matmul(
                            raw[:], rdw_sb[:, dd * 256 + oo * P:dd * 256 + (oo + 1) * P],
                            feats[dd][:], start=(dd == 0), stop=(dd == 3))
                    # stage to outB interleaved (col 2i+oo) for one DMA/block
                    nc.scalar.copy(
                        outB_sb[:, 2 * MOFF[b] + oo:2 * (MOFF[b] + mb):2],
                        raw[:])
                nc.sync.dma_start(
                    out=outB[:, 2 * MOFF[b]:2 * (MOFF[b] + mb)],
                    in_=outB_sb[:, 2 * MOFF[b]:2 * (MOFF[b] + mb)])
            nc.sync.dma_start(out=srow[:, :], in_=srow_sb[:])
    nc.finalize()
    return nc


_PROGRAM = None


def _get_program():
    global _PROGRAM
    if _PROGRAM is None:
        _PROGRAM = _build()
    return _PROGRAM


def _in_maps(exercise_h, kc_h, adj, W1, E, a, rd_w, rd_b):
    f = np.float32
    ex = np.asarray(exercise_h, dtype=np.float64)
    kc = np.asarray(kc_h, dtype=np.float64)
    W1 = np.asarray(W1, dtype=np.float64)
    E_ = np.asarray(E, dtype=np.float64)
    a = np.asarray(a, dtype=np.float64)
    a1, a2 = a[:D, 0], a[D:, 0]

    u = ex @ (W1 @ a1)                        # [N_E]
    vp = np.full(NKC, -60.0)
    vp[:kc.shape[0]] = kc @ (W1 @ a2)
    order = np.argsort(-vp, kind="stable")
    vs = vp[order]
    vmax = vs[0]
    c = float((np.maximum(u + vmax, 0.2 * (u + vmax)) - u).max())

    Brow = (SCALE * np.exp(-0.8 * u - c)).astype(f)            # [N_E]
    Cs = (SCALE * np.exp(vs - c)).astype(f)                    # [NKC]
    Ds = np.exp(0.2 * vs).astype(f)                            # [NKC]
    scal = np.zeros((P, 32), dtype=f)
    scal[:, :16] = Cs.reshape(KCH, P).T
    scal[:, 16:] = Ds.reshape(KCH, P).T

    kcp = np.zeros((NKC, D))
    kcp[:kc.shape[0]] = kc
    kcWh = (kcp[order] @ W1).astype(f)                         # [NKC, D]
    # bf16 stationaries for chunks < NBF: cols kk*256 + t*128 + m
    kcWhTb = np.zeros((P, NBF * 256), dtype=NP_BF16)
    for kk in range(NBF):
        for t in range(2):
            kcWhTb[:, kk * 256 + t * P:kk * 256 + (t + 1) * P] = \
                kcWh[kk * P:(kk + 1) * P, t * P:(t + 1) * P]
    # fp8 DoubleRow stationaries, pairs of chunks (NBF+2pr, NBF+2pr+1)
    kcWh8 = kcWh.astype(NP_FP8)
    kcWhT8 = np.zeros((P, NPAIR * 512), dtype=NP_FP8)
    for pr in range(NPAIR):
        for t in range(2):
            for i in range(2):
                kk = NBF + 2 * pr + i
                kcWhT8[:, pr * 512 + t * 256 + i * P:pr * 512 + t * 256 + (i + 1) * P] = \
                    kcWh8[kk * P:(kk + 1) * P, t * P:(t + 1) * P]

    EmB = np.zeros((P, 2 * 256), dtype=NP_BF16)
    for cc in range(2):
        for d in range(2):
            EmB[:, cc * 256 + d * P:cc * 256 + (d + 1) * P] = \
                E_[cc * P:(cc + 1) * P, d * P:(d + 1) * P]
    rd_w = np.asarray(rd_w, dtype=np.float64)
    rdwB = np.zeros((P, 4 * 256), dtype=NP_BF16)
    for dd in range(4):
        for oo in range(2):
            rdwB[:, dd * 256 + oo * P:dd * 256 + (oo + 1) * P] = \
                rd_w[oo * P:(oo + 1) * P, dd * P:(dd + 1) * P].T

    shared = {"kcWhT8": kcWhT8, "kcWhTb": kcWhTb,
              "EmB": EmB, "rdwB": rdwB, "scal": scal}
    maps = []
    for cidx in range(NCORES):
        sl = slice(cidx * ROWS, (cidx + 1) * ROWS)
        rowB_c = np.zeros((1, M), dtype=NP_BF16)
        rowB_c[0, :ROWS] = Brow[sl]
        rowB_c[0, ROWS:] = np.float32(SCALE * np.exp(-c))
        exTb_c = np.zeros((P, 2 * M), dtype=NP_BF16)
        exv = ex[sl].astype(f)                                 # [ROWS, 256]
        exTb_c[:, :ROWS] = exv[:, :P].T
        exTb_c[:, M:M + ROWS] = exv[:, P:].T
        # adj: reorder kc cols to sorted order (pad kc stay 0), transpose,
        # chunk: adjg[g][p, o*M + i] = adj_sorted[i, (4g+o)*128 + p]
        As = np.zeros((M, NKC), dtype=f)
        real = order < adj.shape[1]
        As[:ROWS, real] = np.asarray(adj[sl], dtype=f)[:, order[real]]
        At = As.T.reshape(KCH, P, M)                           # [kk, p, i]
        m_c = {"rowB": rowB_c, "exTb": exTb_c, **shared}
        for g in range(4):
            ag = np.zeros((P, 4 * M), dtype=NP_FP8)
            for o in range(4):
                ag[:, o * M:(o + 1) * M] = At[g * 4 + o]
            m_c[f"adjg{g}"] = ag
        maps.append(m_c)
    return maps, u, np.asarray(rd_b, dtype=np.float64)


def kernel(exercise_h, kc_h, adj, W1, E, a, rd_w, rd_b):
    nc = _get_program()
    maps, _u, rdb = _in_maps(exercise_h, kc_h, adj, W1, E, a, rd_w, rd_b)
    res = run_bass_kernel_spmd(nc, maps, list(range(NCORES))).results
    out = np.empty((N_E, D), dtype=np.float32)
    for cidx in range(NCORES):
        outB = np.asarray(res[cidx]["outB"]).astype(np.float64)
        s = np.asarray(res[cidx]["srow"]).astype(np.float64)[0, :ROWS]
        A = outB.reshape(P, M, 2)
        raw = np.concatenate([A[:, :ROWS, 0].T, A[:, :ROWS, 1].T], axis=1)
        o = raw / s[:, None] + rdb[None, :]
        o = np.where(o > 0, o, np.expm1(np.minimum(o, 0)))
        out[cidx * ROWS:(cidx + 1) * ROWS] = o.astype(np.float32)
    return out


# revision 11
# speedup vs baseline: 1.7427x; 1.0594x over previous
"""GAT-style graph encoder on 8 trn2 NeuronCores — v2.

Reference (per exercise i over kc nodes j):
    kc_Wh = kc_h @ W1; ex_Wh = ex_h @ W1
    e[i,j] = leaky_relu(u_i + v_j, 0.2),  u = ex_Wh@a1, v = kc_Wh@a2
    att = softmax(where(adj>0, e, -9e15), axis=1)
    new_kc = att @ kc_Wh; ex_Eh = ex_h @ E
    out = elu(concat([new_kc, new_kc*ex_Eh]) @ rd_w.T + rd_b)

Strategy (row-shard exercises over 8 cores, 1250 rows -> padded 1280):
Because the pre-activation logit is separable (u_i + v_j), exp(leaky(s) - r_i)
with the softmax-shift r_i = u_i + c factors into rank-1 products:
    p[j,i] = max(C'_j * 1, D_j * B'_i),  C' = e^{v-c}, D = e^{0.2 v},
    B' = e^{-0.8 u - c}   (host-computed rows; softmax is invariant to any
    per-row scale, so the shift is exact algebra, not an approximation).
On-chip per kc-chunk the whole attention numerator needs just TWO elementwise
passes: a 4x-mode DVE tensor_scalar  (B'_bcast * D_j) max C'_j  and one
mask multiply with adj in {0,1}.  kc nodes are host-sorted by v so the two
top chunks (which carry ~90% of softmax mass) aggregate in bf16 while the
14 tail chunks use fp8e4 DoubleRow matmuls (2 k-tiles per instr at 0.5
cyc/row).  Readout runs in bf16.  The final per-row softmax division, +rd_b
and elu are applied on the host during unshard (per-row scalar epilogue).
"""

import ml_dtypes
import numpy as np

import concourse.bacc as bacc
import concourse.bass as bass
import concourse.mybir as mybir
from concourse.alu_op_type import AluOpType
from concourse.bass_utils import run_bass_kernel_spmd
from concourse.tile import TileContext

F32 = mybir.dt.float32
BF16 = mybir.dt.bfloat16
FP8 = mybir.dt.float8e4
AF = mybir.ActivationFunctionType
DR = mybir.MatmulPerfMode.DoubleRow

P = 128
D = 256
NKC = 2048
KCH = 16                    # kc chunks
NBF = 4                     # leading (high-v) chunks aggregated in bf16
NPAIR = (KCH - NBF) // 2    # fp8 DoubleRow chunk pairs
M = 1280                    # padded exercise rows per core
MBS = (512, 512, 256)
MOFF = (0, 512, 1024)
NCORES = 8
ROWS = 1250
N_E = 10000
SCALE = 128.0               # fp8 range scale folded into B'/C' (cancels in n/s)
# chunks whose mask multiply runs on DVE (fp8 out, 1x) vs Pool
DVE_MASK = frozenset((4, 6, 8, 10, 14))

NP_BF16 = ml_dtypes.bfloat16
NP_FP8 = ml_dtypes.float8_e4m3


def _build():
    nc = bacc.Bacc("TRN2", target_bir_lowering=False, debug=False,
                   num_devices=NCORES)
    adjT2b = nc.declare_dram_parameter("adjT2b", [P, NBF * M], BF16,
                                       isOutput=False)
    adjg = [nc.declare_dram_parameter(f"adjg{g}", [P, 4 * M],
                                      FP8, isOutput=False) for g in range(3)]
    exTb = nc.declare_dram_parameter("exTb", [P, 2 * M], BF16, isOutput=False)
    kcWhT8 = nc.declare_dram_parameter("kcWhT8", [P, NPAIR * 512], FP8,
                                       isOutput=False)
    kcWhTb = nc.declare_dram_parameter("kcWhTb", [P, NBF * 256], BF16,
                                       isOutput=False)
    EmB = nc.declare_dram_parameter("EmB", [P, 2 * 256], BF16, isOutput=False)
    rdwB = nc.declare_dram_parameter("rdwB", [P, 4 * 256], BF16,
                                     isOutput=False)
    rowB = nc.declare_dram_parameter("rowB", [1, M], BF16, isOutput=False)
    scal = nc.declare_dram_parameter("scal", [P, 32], F32, isOutput=False)
    outB = nc.declare_dram_parameter("outB", [P, 2 * M], BF16, isOutput=True)
    srow = nc.declare_dram_parameter("srow", [1, M], F32, isOutput=True)

    with TileContext(nc) as tc:
        with tc.tile_pool(name="const", bufs=1) as cpool, \
             tc.tile_pool(name="acc_ps", bufs=2, space="PSUM") as apool, \
             tc.tile_pool(name="out_ps", bufs=2, space="PSUM") as opool, \
             tc.tile_pool(name="mwork", bufs=6) as mpool, \
             tc.tile_pool(name="post", bufs=2) as qpool:
            # ---- const loads, ordered so small gating tiles land first
            scal_sb = cpool.tile([P, 32], F32, tag="scal")
            nc.sync.dma_start(out=scal_sb[:], in_=scal[:, :])
            rowB_sb = cpool.tile([1, M], BF16, tag="rowB")
            nc.sync.dma_start(out=rowB_sb[:], in_=rowB[:, :])
            kcb_sb = cpool.tile([P, NBF * 256], BF16, tag="kcWhTb")
            nc.sync.dma_start(out=kcb_sb[:], in_=kcWhTb[:, :])
            adjT2_sb = cpool.tile([P, NBF * M], BF16, tag="adjT2b")
            nc.sync.dma_start(out=adjT2_sb[:], in_=adjT2b[:, :])
            exT_sb = cpool.tile([P, 2 * M], BF16, tag="exTb")
            nc.sync.dma_start(out=exT_sb[:], in_=exTb[:, :])
            # 4-D: [p, (pair,target), ktile=2, m] so DoubleRow APs expose Num=2
            kc8_sb = cpool.tile([P, NPAIR * 2, 2, P], FP8, tag="kcWhT8")
            nc.sync.dma_start(out=kc8_sb[:], in_=kcWhT8[:, :])
            EmB_sb = cpool.tile([P, 2 * 256], BF16, tag="EmB")
            nc.sync.dma_start(out=EmB_sb[:], in_=EmB[:, :])
            rdw_sb = cpool.tile([P, 4 * 256], BF16, tag="rdwB")
            nc.sync.dma_start(out=rdw_sb[:], in_=rdwB[:, :])
            adj_sb = []
            for g in range(3):
                t = cpool.tile([P, 4 * M], FP8, tag=f"adjg{g}", name=f"adjg{g}")
                nc.sync.dma_start(out=t[:], in_=adjg[g][:, :])
                adj_sb.append(t)

            ones1b = cpool.tile([1, P], BF16, tag="ones1b")
            nc.vector.memset(ones1b[:], 1.0)
            onesb = cpool.tile([P, 1], BF16, tag="onesb")
            nc.vector.memset(onesb[:], 1.0)
            ones8 = cpool.tile([P, 2, 16], FP8, tag="ones8")
            nc.vector.memset(ones8[:], 1.0)

            Bb = cpool.tile([P, M], BF16, tag="Bb")          # B' broadcast
            exEhT = [cpool.tile([P, M], BF16, tag=f"exEhT{d}", name=f"exEhT{d}")
                     for d in (0, 1)]
            outB_sb = cpool.tile([P, 2 * M], BF16, tag="outB_sb")
            srow_sb = cpool.tile([1, M], F32, tag="srow_sb")

            # ---- setup: B' broadcast + ex_Eh (bf16 matmuls, ACT copies)
            # setup psum shares the readout pool (tag "raw"), disjoint in time
            for b in range(3):
                ms = slice(MOFF[b], MOFF[b] + MBS[b])
                ps = opool.tile([P, MBS[b]], F32, tag="raw", name=f"bb_ps{b}")
                nc.tensor.matmul(ps[:], ones1b[:], rowB_sb[:, ms],
                                 start=True, stop=True)
                nc.scalar.copy(Bb[:, ms], ps[:])
            for d in range(2):
                for b in range(3):
                    ms = slice(MOFF[b], MOFF[b] + MBS[b])
                    ps = opool.tile([P, MBS[b]], F32, tag="raw",
                                    name=f"eh_ps{d}_{b}")
                    for c in range(2):
                        nc.tensor.matmul(
                            ps[:],
                            EmB_sb[:, c * 256 + d * P:c * 256 + (d + 1) * P],
                            exT_sb[:, c * M + MOFF[b]:c * M + MOFF[b] + MBS[b]],
                            start=(c == 0), stop=(c == 1))
                    nc.scalar.copy(exEhT[d][:, ms], ps[:])

            # ---- full-width masked-exp per chunk (q: one 4x DVE ts; mask:
            # mult with {0,1} adj; bf16 chunks on DVE 2x, fp8 split DVE/Pool)
            def adjsl(kk):
                if kk < NBF:
                    return adjT2_sb[:, kk * M:(kk + 1) * M]
                g, o = divmod(kk - NBF, 4)
                return adj_sb[g][:, o * M:(o + 1) * M]

            def q_of(kk):
                q = mpool.tile([P, M], BF16, tag="q", bufs=4)
                # (B'_i * D_j) max C'_j : whole unmasked exp in one 4x op
                nc.vector.tensor_scalar(
                    q[:], Bb[:], scal_sb[:, 16 + kk:17 + kk],
                    scal_sb[:, kk:kk + 1], AluOpType.mult, AluOpType.max)
                return q

            ptmb = []
            for kk in range(NBF):
                q = q_of(kk)
                t = mpool.tile([P, M], BF16, tag="ptmb", bufs=NBF,
                               name=f"ptmb{kk}")
                nc.vector.tensor_mul(t[:], q[:], adjsl(kk))
                ptmb.append(t)
            ptm8 = []
            for pr in range(NPAIR):
                t8 = mpool.tile([P, 2, M], FP8, tag="ptm8", bufs=NPAIR,
                                name=f"ptm8_{pr}")
                for h in range(2):
                    kk = NBF + 2 * pr + h
                    q = q_of(kk)
                    eng = nc.vector if kk in DVE_MASK else nc.gpsimd
                    eng.tensor_mul(t8[:, h, :], q[:], adjsl(kk))
                ptm8.append(t8)

            # ---- per m-block aggregation + readout
            for b in range(3):
                mb = MBS[b]
                ms = slice(MOFF[b], MOFF[b] + mb)
                n0 = apool.tile([P, mb], F32, tag="n0")
                n1 = apool.tile([P, mb], F32, tag="n1")
                sS = apool.tile([1, mb], F32, tag="sS")
                for kk in range(NBF):
                    st = (kk == 0)
                    nc.tensor.matmul(n0[:], kcb_sb[:, kk * 256:kk * 256 + P],
                                     ptmb[kk][:, ms], start=st, stop=False)
                    nc.tensor.matmul(n1[:], kcb_sb[:, kk * 256 + P:(kk + 1) * 256],
                                     ptmb[kk][:, ms], start=st, stop=False)
                    nc.tensor.matmul(sS[:], onesb[:], ptmb[kk][:, ms],
                                     start=st, stop=False)
                for pr in range(NPAIR):
                    sp = (pr == NPAIR - 1)
                    nc.tensor.matmul(n0[:], kc8_sb[:, 2 * pr, :, :],
                                     ptm8[pr][:, :, ms], start=False, stop=sp,
                                     perf_mode=DR)
                    nc.tensor.matmul(n1[:], kc8_sb[:, 2 * pr + 1, :, :],
                                     ptm8[pr][:, :, ms], start=False, stop=sp,
                                     perf_mode=DR)
                    nc.tensor.matmul(sS[:], ones8[:, :, 0:1],
                                     ptm8[pr][:, :, ms], start=False, stop=sp,
                                     perf_mode=DR)

                # ---- post: features, readout, stage out
                nc.vector.tensor_copy(srow_sb[:, ms], sS[:])
                ncf = []
                for t in range(2):
                    nt = qpool.tile([P, mb], BF16, tag=f"nc{t}", name=f"nc{t}")
                    nc.scalar.copy(nt[:], (n0 if t == 0 else n1)[:])
                    ncf.append(nt)
                tf = []
                for t in range(2):
                    tt = qpool.tile([P, mb], BF16, tag=f"t{t}", name=f"tt{t}")
                    nc.vector.tensor_mul(tt[:], ncf[t][:], exEhT[t][:, ms])
                    tf.append(tt)
                feats = [ncf[0], ncf[1], tf[0], tf[1]]
                for oo in range(2):
                    raw = opool.tile([P, mb], F32, tag="raw")
                    for dd in range(4):
                        nc.tensor.matmul(
                            raw[:], rdw_sb[:, dd * 256 + oo * P:dd * 256 + (oo + 1) * P],
                            feats[dd][:], start=(dd == 0), stop=(dd == 3))
                    # stage to outB interleaved (col 2i+oo) for one DMA/block
                    nc.scalar.copy(
                        outB_sb[:, 2 * MOFF[b] + oo:2 * (MOFF[b] + mb):2],
                        raw[:])
                nc.sync.dma_start(
                    out=outB[:, 2 * MOFF[b]:2 * (MOFF[b] + mb)],
                    in_=outB_sb[:, 2 * MOFF[b]:2 * (MOFF[b] + mb)])
            nc.sync.dma_start(out=srow[:, :], in_=srow_sb[:])
    nc.finalize()
    return nc


_PROGRAM = None


def _get_program():
    global _PROGRAM
    if _PROGRAM is None:
        _PROGRAM = _build()
    return _PROGRAM


def _in_maps(exercise_h, kc_h, adj, W1, E, a, rd_w, rd_b):
    f = np.float32
    ex = np.asarray(exercise_h, dtype=np.float64)
    kc = np.asarray(kc_h, dtype=np.float64)
    W1 = np.asarray(W1, dtype=np.float64)
    E_ = np.asarray(E, dtype=np.float64)
    a = np.asarray(a, dtype=np.float64)
    a1, a2 = a[:D, 0], a[D:, 0]

    u = ex @ (W1 @ a1)                        # [N_E]
    vp = np.full(NKC, -60.0)
    vp[:kc.shape[0]] = kc @ (W1 @ a2)
    order = np.argsort(-vp, kind="stable")
    vs = vp[order]
    vmax = vs[0]
    c = float((np.maximum(u + vmax, 0.2 * (u + vmax)) - u).max())

    Brow = (SCALE * np.exp(-0.8 * u - c)).astype(f)            # [N_E]
    Cs = (SCALE * np.exp(vs - c)).astype(f)                    # [NKC]
    Ds = np.exp(0.2 * vs).astype(f)                            # [NKC]
    scal = np.zeros((P, 32), dtype=f)
    scal[:, :16] = Cs.reshape(KCH, P).T
    scal[:, 16:] = Ds.reshape(KCH, P).T

    kcp = np.zeros((NKC, D))
    kcp[:kc.shape[0]] = kc
    kcWh = (kcp[order] @ W1).astype(f)                         # [NKC, D]
    # bf16 stationaries for chunks < NBF: cols kk*256 + t*128 + m
    kcWhTb = np.zeros((P, NBF * 256), dtype=NP_BF16)
    for kk in range(NBF):
        for t in range(2):
            kcWhTb[:, kk * 256 + t * P:kk * 256 + (t + 1) * P] = \
                kcWh[kk * P:(kk + 1) * P, t * P:(t + 1) * P]
    # fp8 DoubleRow stationaries, pairs of chunks (NBF+2pr, NBF+2pr+1)
    kcWh8 = kcWh.astype(NP_FP8)
    kcWhT8 = np.zeros((P, NPAIR * 512), dtype=NP_FP8)
    for pr in range(NPAIR):
        for t in range(2):
            for i in range(2):
                kk = NBF + 2 * pr + i
                kcWhT8[:, pr * 512 + t * 256 + i * P:pr * 512 + t * 256 + (i + 1) * P] = \
                    kcWh8[kk * P:(kk + 1) * P, t * P:(t + 1) * P]

    EmB = np.zeros((P, 2 * 256), dtype=NP_BF16)
    for cc in range(2):
        for d in range(2):
            EmB[:, cc * 256 + d * P:cc * 256 + (d + 1) * P] = \
                E_[cc * P:(cc + 1) * P, d * P:(d + 1) * P]
    rd_w = np.asarray(rd_w, dtype=np.float64)
    rdwB = np.zeros((P, 4 * 256), dtype=NP_BF16)
    for dd in range(4):
        for oo in range(2):
            rdwB[:, dd * 256 + oo * P:dd * 256 + (oo + 1) * P] = \
                rd_w[oo * P:(oo + 1) * P, dd * P:(dd + 1) * P].T

    shared = {"kcWhT8": kcWhT8, "kcWhTb": kcWhTb,
              "EmB": EmB, "rdwB": rdwB, "scal": scal}
    maps = []
    for cidx in range(NCORES):
        sl = slice(cidx * ROWS, (cidx + 1) * ROWS)
        rowB_c = np.zeros((1, M), dtype=NP_BF16)
        rowB_c[0, :ROWS] = Brow[sl]
        rowB_c[0, ROWS:] = np.float32(SCALE * np.exp(-c))
        exTb_c = np.zeros((P, 2 * M), dtype=NP_BF16)
        exv = ex[sl].astype(f)                                 # [ROWS, 256]
        exTb_c[:, :ROWS] = exv[:, :P].T
        exTb_c[:, M:M + ROWS] = exv[:, P:].T
        # adj: reorder kc cols to sorted order (pad kc stay 0), transpose,
        # chunk, min-encode mask: keep -> BIG (>= any q), drop -> 0
        As = np.zeros((M, NKC), dtype=f)
        real = order < adj.shape[1]
        As[:ROWS, real] = np.asarray(adj[sl], dtype=f)[:, order[real]]
        At = As.T.reshape(KCH, P, M)                           # [kk, p, i]
        m_c = {"rowB": rowB_c, "exTb": exTb_c, **shared}
        a2b = np.zeros((P, NBF * M), dtype=NP_BF16)
        for kk in range(NBF):
            a2b[:, kk * M:(kk + 1) * M] = At[kk]
        m_c["adjT2b"] = a2b
        for g in range(3):
            ag = np.zeros((P, 4 * M), dtype=NP_FP8)
            for o in range(4):
                ag[:, o * M:(o + 1) * M] = At[NBF + g * 4 + o]
            m_c[f"adjg{g}"] = ag
        maps.append(m_c)
    return maps, u, np.asarray(rd_b, dtype=np.float64)


def kernel(exercise_h, kc_h, adj, W1, E, a, rd_w, rd_b):
    nc = _get_program()
    maps, _u, rdb = _in_maps(exercise_h, kc_h, adj, W1, E, a, rd_w, rd_b)
    res = run_bass_kernel_spmd(nc, maps, list(range(NCORES))).results
    out = np.empty((N_E, D), dtype=np.float32)
    for cidx in range(NCORES):
        outB = np.asarray(res[cidx]["outB"]).astype(np.float64)
        s = np.asarray(res[cidx]["srow"]).astype(np.float64)[0, :ROWS]
        A = outB.reshape(P, M, 2)
        raw = np.concatenate([A[:, :ROWS, 0].T, A[:, :ROWS, 1].T], axis=1)
        o = raw / s[:, None] + rdb[None, :]
        o = np.where(o > 0, o, np.expm1(np.minimum(o, 0)))
        out[cidx * ROWS:(cidx + 1) * ROWS] = o.astype(np.float32)
    return out


# revision 14
# speedup vs baseline: 1.8387x; 1.0551x over previous
"""GAT-style graph encoder on 8 trn2 NeuronCores — v3.

Reference (per exercise i over kc nodes j):
    kc_Wh = kc_h @ W1; ex_Wh = ex_h @ W1
    e[i,j] = leaky_relu(u_i + v_j, 0.2),  u = ex_Wh@a1, v = kc_Wh@a2
    att = softmax(where(adj>0, e, -9e15), axis=1)
    new_kc = att @ kc_Wh; ex_Eh = ex_h @ E
    out = elu(concat([new_kc, new_kc*ex_Eh]) @ rd_w.T + rd_b)

Strategy (row-shard exercises over 8 cores, 1250 rows -> padded 1280):
The pre-activation logit is separable (u_i + v_j), so with the softmax shift
r_i = u_i + c (softmax is invariant to any per-row scale) the masked exp
factors into rank-1 products:
    p[j,i] = adj * max(C'_j, D_j * B'_i),  C' = e^{v-c}, D = e^{0.2 v},
    B' = e^{-0.8 u - c}    (all host-computed rows; exact algebra).
kc nodes are host-sorted by v (descending), exercises are host-sorted by u
(descending, per core).  Then for each kc chunk there is a column prefix
t_kk = #{i : u_i >= -min_j v_j} where the positive branch wins for EVERY
(j,i) pair, i.e. p = adj * C'_j exactly.  For that prefix the aggregation is
a plain matmul with adj itself as the moving tensor and kcWh*C' folded into
the stationary - no elementwise work at all.  Only the column suffix needs
the two elementwise passes (a 4x DVE tensor_scalar for q and a mask multiply
split across DVE/Pool).  The 4 top (high-v) chunks aggregate in bf16; the 12
tail chunks use fp8e4 DoubleRow matmuls (2 k-tiles/instr at 0.5 cyc/row).
Readout runs in bf16.  The per-row softmax division, +rd_b and elu are
applied on the host during unshard (per-row scalar epilogue).
"""

import ml_dtypes
import numpy as np

import concourse.bacc as bacc
import concourse.bass as bass
import concourse.mybir as mybir
from concourse.alu_op_type import AluOpType
from concourse.bass_utils import run_bass_kernel_spmd
from concourse.tile import TileContext

F32 = mybir.dt.float32
BF16 = mybir.dt.bfloat16
FP8 = mybir.dt.float8e4
DR = mybir.MatmulPerfMode.DoubleRow

P = 128
D = 256
NKC = 2048
KCH = 16                    # kc chunks
NBF = 4                     # leading (high-v) chunks aggregated in bf16
NPAIR = (KCH - NBF) // 2    # fp8 DoubleRow chunk pairs
M = 1280                    # padded exercise rows per core
MBS = (512, 512, 256)
MOFF = (0, 512, 1024)
NCORES = 8
ROWS = 1250
N_E = 10000
SCALE = 128.0               # fp8 range scale folded into B'/C' (cancels in n/s)
# tail chunks whose suffix mask multiply runs on DVE (fp8 out, 1x) vs Pool
DVE_MASK = frozenset((4, 6, 8, 10, 14))

NP_BF16 = ml_dtypes.bfloat16
NP_FP8 = ml_dtypes.float8_e4m3


def _build(Ts):
    """Ts: per-chunk column counts (multiple of 64) where p = adj*C' exactly."""
    nc = bacc.Bacc("TRN2", target_bir_lowering=False, debug=False,
                   num_devices=NCORES)
    adjTb = nc.declare_dram_parameter("adjTb", [P, NBF * M], BF16,
                                      isOutput=False)
    adjg = [nc.declare_dram_parameter(f"adjg{g}", [P, 4 * M],
                                      FP8, isOutput=False) for g in range(3)]
    exTb = nc.declare_dram_parameter("exTb", [P, 2 * M], BF16, isOutput=False)
    kcWhT8 = nc.declare_dram_parameter("kcWhT8", [P, NPAIR * 512], FP8,
                                       isOutput=False)
    kcCT8 = nc.declare_dram_parameter("kcCT8", [P, NPAIR * 512], FP8,
                                      isOutput=False)
    sC8 = nc.declare_dram_parameter("sC8", [P, NPAIR * 32], FP8,
                                    isOutput=False)
    kcWhTb = nc.declare_dram_parameter("kcWhTb", [P, NBF * 256], BF16,
                                       isOutput=False)
    kcCTb = nc.declare_dram_parameter("kcCTb", [P, NBF * 256], BF16,
                                      isOutput=False)
    sCb = nc.declare_dram_parameter("sCb", [P, NBF], BF16, isOutput=False)
    EmB = nc.declare_dram_parameter("EmB", [P, 2 * 256], BF16, isOutput=False)
    rdwB = nc.declare_dram_parameter("rdwB", [P, 4 * 256], BF16,
                                     isOutput=False)
    rowB = nc.declare_dram_parameter("rowB", [1, M], BF16, isOutput=False)
    scal = nc.declare_dram_parameter("scal", [P, 32], F32, isOutput=False)
    outB = nc.declare_dram_parameter("outB", [P, 2 * M], BF16, isOutput=True)
    srow = nc.declare_dram_parameter("srow", [1, M], F32, isOutput=True)

    Tpair = [min(Ts[NBF + 2 * pr], Ts[NBF + 2 * pr + 1])
             for pr in range(NPAIR)]

    with TileContext(nc) as tc:
        with tc.tile_pool(name="const", bufs=1) as cpool, \
             tc.tile_pool(name="acc_ps", bufs=2, space="PSUM") as apool, \
             tc.tile_pool(name="out_ps", bufs=2, space="PSUM") as opool, \
             tc.tile_pool(name="mwork", bufs=4) as mpool, \
             tc.tile_pool(name="post", bufs=2) as qpool:
            # ---- const loads, ordered so gating tiles land first
            scal_sb = cpool.tile([P, 32], F32, tag="scal")
            nc.sync.dma_start(out=scal_sb[:], in_=scal[:, :])
            rowB_sb = cpool.tile([1, M], BF16, tag="rowB")
            nc.sync.dma_start(out=rowB_sb[:], in_=rowB[:, :])
            kcb_sb = cpool.tile([P, NBF * 256], BF16, tag="kcWhTb")
            nc.sync.dma_start(out=kcb_sb[:], in_=kcWhTb[:, :])
            kcCb_sb = cpool.tile([P, NBF * 256], BF16, tag="kcCTb")
            nc.sync.dma_start(out=kcCb_sb[:], in_=kcCTb[:, :])
            sCb_sb = cpool.tile([P, NBF], BF16, tag="sCb")
            nc.sync.dma_start(out=sCb_sb[:], in_=sCb[:, :])
            adjT4_sb = cpool.tile([P, NBF, M], BF16, tag="adjTb")
            nc.sync.dma_start(out=adjT4_sb[:], in_=adjTb[:, :])
            # 4-D: [p, (pair,target), ktile=2, m] so DoubleRow APs expose Num=2
            kc8_sb = cpool.tile([P, NPAIR * 2, 2, P], FP8, tag="kcWhT8")
            nc.sync.dma_start(out=kc8_sb[:], in_=kcWhT8[:, :])
            kcC8_sb = cpool.tile([P, NPAIR * 2, 2, P], FP8, tag="kcCT8")
            nc.sync.dma_start(out=kcC8_sb[:], in_=kcCT8[:, :])
            sC8_sb = cpool.tile([P, NPAIR, 2, 16], FP8, tag="sC8")
            nc.sync.dma_start(out=sC8_sb[:], in_=sC8[:, :])
            exT_sb = cpool.tile([P, 2 * M], BF16, tag="exTb")
            nc.sync.dma_start(out=exT_sb[:], in_=exTb[:, :])
            EmB_sb = cpool.tile([P, 2 * 256], BF16, tag="EmB")
            nc.sync.dma_start(out=EmB_sb[:], in_=EmB[:, :])
            rdw_sb = cpool.tile([P, 4 * 256], BF16, tag="rdwB")
            nc.sync.dma_start(out=rdw_sb[:], in_=rdwB[:, :])
            adj_sb = []
            for g in range(3):
                t = cpool.tile([P, 4, M], FP8, tag=f"adjg{g}", name=f"adjg{g}")
                nc.sync.dma_start(out=t[:], in_=adjg[g][:, :])
                adj_sb.append(t)

            ones1b = cpool.tile([1, P], BF16, tag="ones1b")
            nc.vector.memset(ones1b[:], 1.0)
            onesb = cpool.tile([P, 1], BF16, tag="onesb")
            nc.vector.memset(onesb[:], 1.0)
            ones8 = cpool.tile([P, 2, 16], FP8, tag="ones8")
            nc.vector.memset(ones8[:], 1.0)

            Bb = cpool.tile([P, M], BF16, tag="Bb")          # B' broadcast
            exEhT = [cpool.tile([P, M], BF16, tag=f"exEhT{d}", name=f"exEhT{d}")
                     for d in (0, 1)]
            outB_sb = cpool.tile([P, 2 * M], BF16, tag="outB_sb")
            srow_sb = cpool.tile([1, M], F32, tag="srow_sb")

            # ---- setup: B' broadcast + ex_Eh (psum shared with readout pool)
            for b in range(3):
                ms = slice(MOFF[b], MOFF[b] + MBS[b])
                ps = opool.tile([P, MBS[b]], F32, tag="raw", name=f"bb_ps{b}")
                nc.tensor.matmul(ps[:], ones1b[:], rowB_sb[:, ms],
                                 start=True, stop=True)
                nc.scalar.copy(Bb[:, ms], ps[:])
            for d in range(2):
                for b in range(3):
                    ms = slice(MOFF[b], MOFF[b] + MBS[b])
                    ps = opool.tile([P, MBS[b]], F32, tag="raw",
                                    name=f"eh_ps{d}_{b}")
                    for c in range(2):
                        nc.tensor.matmul(
                            ps[:],
                            EmB_sb[:, c * 256 + d * P:c * 256 + (d + 1) * P],
                            exT_sb[:, c * M + MOFF[b]:c * M + MOFF[b] + MBS[b]],
                            start=(c == 0), stop=(c == 1))
                    nc.scalar.copy(exEhT[d][:, ms], ps[:])

            # ---- suffix-only masked-exp (cols >= T of each chunk)
            def adjsl(kk, lo, hi):
                if kk < NBF:
                    return adjT4_sb[:, kk, lo:hi]
                g, o = divmod(kk - NBF, 4)
                return adj_sb[g][:, o, lo:hi]

            def q_of(kk, t0):
                q = mpool.tile([P, M], BF16, tag="q", bufs=4,
                               name=f"q{kk}")
                # (B'_i * D_j) max C'_j : whole unmasked exp in one 4x op
                nc.vector.tensor_scalar(
                    q[:, t0:], Bb[:, t0:], scal_sb[:, 16 + kk:17 + kk],
                    scal_sb[:, kk:kk + 1], AluOpType.mult, AluOpType.max)
                return q

            ptmb = []
            for kk in range(NBF):
                t0 = 0 if kk == 0 else Ts[kk]
                if t0 >= M:
                    ptmb.append(None)
                    continue
                q = q_of(kk, t0)
                t = mpool.tile([P, M], BF16, tag="ptmb", bufs=NBF,
                               name=f"ptmb{kk}")
                nc.vector.tensor_mul(t[:, t0:], q[:, t0:], adjsl(kk, t0, M))
                ptmb.append(t)
            ptm8 = []
            for pr in range(NPAIR):
                t0 = Tpair[pr]
                if t0 >= M:
                    ptm8.append(None)
                    continue
                t8 = mpool.tile([P, 2, M], FP8, tag="ptm8", bufs=NPAIR,
                                name=f"ptm8_{pr}")
                for h in range(2):
                    kk = NBF + 2 * pr + h
                    q = q_of(kk, t0)
                    eng = nc.vector if kk in DVE_MASK else nc.gpsimd
                    eng.tensor_mul(t8[:, h, t0:], q[:, t0:], adjsl(kk, t0, M))
                ptm8.append(t8)

            # ---- per m-block aggregation + readout
            for b in range(3):
                mb = MBS[b]
                ms = slice(MOFF[b], MOFF[b] + mb)
                n0 = apool.tile([P, mb], F32, tag="n0")
                n1 = apool.tile([P, mb], F32, tag="n1")
                sS = apool.tile([1, mb], F32, tag="sS")

                # bf16 chunks: A-prefix via adj-matmul, B-suffix via ptm.
                # chunk 0 runs full-width (start=True zeroes the whole 2KB
                # psum bank, so the group opener must cover the bank alone)
                for kk in range(NBF):
                    aw = 0 if kk == 0 else min(max(Ts[kk] - MOFF[b], 0), mb)
                    st = (kk == 0)
                    if aw > 0:
                        asl = adjsl(kk, MOFF[b], MOFF[b] + aw)
                        nc.tensor.matmul(
                            n0[:, 0:aw], kcCb_sb[:, kk * 256:kk * 256 + P],
                            asl, start=st, stop=False, skip_group_check=True)
                        nc.tensor.matmul(
                            n1[:, 0:aw], kcCb_sb[:, kk * 256 + P:(kk + 1) * 256],
                            asl, start=st, stop=False, skip_group_check=True)
                        nc.tensor.matmul(
                            sS[:, 0:aw], sCb_sb[:, kk:kk + 1],
                            asl, start=st, stop=False, skip_group_check=True)
                    if aw < mb:
                        pm = ptmb[kk][:, MOFF[b] + aw:MOFF[b] + mb]
                        nc.tensor.matmul(
                            n0[:, aw:mb], kcb_sb[:, kk * 256:kk * 256 + P],
                            pm, start=st, stop=False, skip_group_check=True)
                        nc.tensor.matmul(
                            n1[:, aw:mb], kcb_sb[:, kk * 256 + P:(kk + 1) * 256],
                            pm, start=st, stop=False, skip_group_check=True)
                        nc.tensor.matmul(
                            sS[:, aw:mb], onesb[:],
                            pm, start=st, stop=False, skip_group_check=True)
                # fp8 DoubleRow pairs
                for pr in range(NPAIR):
                    aw = min(max(Tpair[pr] - MOFF[b], 0), mb)
                    g, o = divmod(2 * pr, 4)
                    sp = (pr == NPAIR - 1)
                    if aw > 0:
                        adjpair = adj_sb[g][:, o:o + 2, MOFF[b]:MOFF[b] + aw]
                        nc.tensor.matmul(
                            n0[:, 0:aw], kcC8_sb[:, 2 * pr, :, :], adjpair,
                            start=False, stop=sp and aw >= mb,
                            perf_mode=DR, skip_group_check=True)
                        nc.tensor.matmul(
                            n1[:, 0:aw], kcC8_sb[:, 2 * pr + 1, :, :], adjpair,
                            start=False, stop=sp and aw >= mb,
                            perf_mode=DR, skip_group_check=True)
                        nc.tensor.matmul(
                            sS[:, 0:aw], sC8_sb[:, pr, :, 0:1], adjpair,
                            start=False, stop=sp and aw >= mb,
                            perf_mode=DR, skip_group_check=True)
                    if aw < mb:
                        pm = ptm8[pr][:, :, MOFF[b] + aw:MOFF[b] + mb]
                        nc.tensor.matmul(
                            n0[:, aw:mb], kc8_sb[:, 2 * pr, :, :], pm,
                            start=False, stop=sp, perf_mode=DR,
                            skip_group_check=True)
                        nc.tensor.matmul(
                            n1[:, aw:mb], kc8_sb[:, 2 * pr + 1, :, :], pm,
                            start=False, stop=sp, perf_mode=DR,
                            skip_group_check=True)
                        nc.tensor.matmul(
                            sS[:, aw:mb], ones8[:, :, 0:1], pm,
                            start=False, stop=sp, perf_mode=DR,
                            skip_group_check=True)

                # ---- post: features, readout, stage out
                nc.vector.tensor_copy(srow_sb[:, ms], sS[:])
                ncf = []
                for t in range(2):
                    nt = qpool.tile([P, mb], BF16, tag=f"nc{t}", name=f"nc{t}")
                    nc.scalar.copy(nt[:], (n0 if t == 0 else n1)[:])
                    ncf.append(nt)
                tf = []
                for t in range(2):
                    tt = qpool.tile([P, mb], BF16, tag=f"t{t}", name=f"tt{t}")
                    nc.vector.tensor_mul(tt[:], ncf[t][:], exEhT[t][:, ms])
                    tf.append(tt)
                feats = [ncf[0], ncf[1], tf[0], tf[1]]
                for oo in range(2):
                    raw = opool.tile([P, mb], F32, tag="raw")
                    for dd in range(4):
                        nc.tensor.matmul(
                            raw[:], rdw_sb[:, dd * 256 + oo * P:dd * 256 + (oo + 1) * P],
                            feats[dd][:], start=(dd == 0), stop=(dd == 3))
                    # stage to outB interleaved (col 2i+oo) for one DMA/block
                    nc.scalar.copy(
                        outB_sb[:, 2 * MOFF[b] + oo:2 * (MOFF[b] + mb):2],
                        raw[:])
                nc.sync.dma_start(
                    out=outB[:, 2 * MOFF[b]:2 * (MOFF[b] + mb)],
                    in_=outB_sb[:, 2 * MOFF[b]:2 * (MOFF[b] + mb)])
            nc.sync.dma_start(out=srow[:, :], in_=srow_sb[:])
    nc.finalize()
    return nc


_PROGRAMS = {}


def _get_program(Ts):
    key = tuple(Ts)
    if key not in _PROGRAMS:
        _PROGRAMS[key] = _build(key)
    return _PROGRAMS[key]


def _prep(exercise_h, kc_h, adj, W1, E, a, rd_w, rd_b):
    f = np.float32
    ex = np.asarray(exercise_h, dtype=np.float64)
    kc = np.asarray(kc_h, dtype=np.float64)
    W1 = np.asarray(W1, dtype=np.float64)
    E_ = np.asarray(E, dtype=np.float64)
    a = np.asarray(a, dtype=np.float64)
    a1, a2 = a[:D, 0], a[D:, 0]

    u = ex @ (W1 @ a1)                        # [N_E]
    vp = np.full(NKC, -60.0)
    vp[:kc.shape[0]] = kc @ (W1 @ a2)
    order = np.argsort(-vp, kind="stable")
    vs = vp[order]
    vmax = vs[0]
    c = float((np.maximum(u + vmax, 0.2 * (u + vmax)) - u).max())

    Brow = (SCALE * np.exp(-0.8 * u - c)).astype(f)            # [N_E]
    Cs = (SCALE * np.exp(vs - c)).astype(f)                    # [NKC]
    Ds = np.exp(0.2 * vs).astype(f)                            # [NKC]
    scal = np.zeros((P, 32), dtype=f)
    scal[:, :16] = Cs.reshape(KCH, P).T
    scal[:, 16:] = Ds.reshape(KCH, P).T

    # per-core exercise sort by u (descending) + per-chunk exact-C prefix
    perms = []
    Ts = np.full(KCH, M, dtype=np.int64)
    vlo = vs.reshape(KCH, P).min(axis=1)                       # chunk min v
    for cidx in range(NCORES):
        uc = u[cidx * ROWS:(cidx + 1) * ROWS]
        perm = np.argsort(-uc, kind="stable")
        perms.append(perm)
        us = uc[perm]
        for kk in range(KCH):
            cnt = int((us >= -vlo[kk]).sum())                  # prefix length
            Ts[kk] = min(Ts[kk], cnt)
    Ts = (Ts // 64) * 64                                       # align, pads are B-cols
    Ts = np.minimum(Ts, ROWS // 64 * 64)

    kcp = np.zeros((NKC, D))
    kcp[:kc.shape[0]] = kc
    kcWh = (kcp[order] @ W1).astype(f)                         # [NKC, D]
    kcC = (kcWh * Cs[:, None]).astype(f)                       # C'-folded

    def stat_b(src):
        out = np.zeros((P, NBF * 256), dtype=NP_BF16)
        for kk in range(NBF):
            for t in range(2):
                out[:, kk * 256 + t * P:kk * 256 + (t + 1) * P] = \
                    src[kk * P:(kk + 1) * P, t * P:(t + 1) * P]
        return out

    def stat_8(src):
        s8 = src.astype(NP_FP8)
        out = np.zeros((P, NPAIR * 512), dtype=NP_FP8)
        for pr in range(NPAIR):
            for t in range(2):
                for i in range(2):
                    kk = NBF + 2 * pr + i
                    out[:, pr * 512 + t * 256 + i * P:pr * 512 + t * 256 + (i + 1) * P] = \
                        s8[kk * P:(kk + 1) * P, t * P:(t + 1) * P]
        return out

    kcWhTb = stat_b(kcWh)
    kcCTb = stat_b(kcC)
    kcWhT8 = stat_8(kcWh)
    kcCT8 = stat_8(kcC)
    sCb = np.zeros((P, NBF), dtype=NP_BF16)
    for kk in range(NBF):
        sCb[:, kk] = Cs[kk * P:(kk + 1) * P]
    sC8 = np.zeros((P, NPAIR * 32), dtype=NP_FP8)
    for pr in range(NPAIR):
        for i in range(2):
            kk = NBF + 2 * pr + i
            sC8[:, pr * 32 + i * 16] = Cs[kk * P:(kk + 1) * P]

    EmB = np.zeros((P, 2 * 256), dtype=NP_BF16)
    for cc in range(2):
        for d in range(2):
            EmB[:, cc * 256 + d * P:cc * 256 + (d + 1) * P] = \
                E_[cc * P:(cc + 1) * P, d * P:(d + 1) * P]
    rd_w = np.asarray(rd_w, dtype=np.float64)
    rdwB = np.zeros((P, 4 * 256), dtype=NP_BF16)
    for dd in range(4):
        for oo in range(2):
            rdwB[:, dd * 256 + oo * P:dd * 256 + (oo + 1) * P] = \
                rd_w[oo * P:(oo + 1) * P, dd * P:(dd + 1) * P].T

    shared = {"kcWhT8": kcWhT8, "kcCT8": kcCT8, "sC8": sC8,
              "kcWhTb": kcWhTb, "kcCTb": kcCTb, "sCb": sCb,
              "EmB": EmB, "rdwB": rdwB, "scal": scal}
    maps = []
    for cidx in range(NCORES):
        sl = slice(cidx * ROWS, (cidx + 1) * ROWS)
        perm = perms[cidx]
        rowB_c = np.zeros((1, M), dtype=NP_BF16)
        rowB_c[0, :ROWS] = Brow[sl][perm]
        rowB_c[0, ROWS:] = np.float32(SCALE * np.exp(-c))
        exTb_c = np.zeros((P, 2 * M), dtype=NP_BF16)
        exv = ex[sl].astype(f)[perm]                           # [ROWS, 256]
        exTb_c[:, :ROWS] = exv[:, :P].T
        exTb_c[:, M:M + ROWS] = exv[:, P:].T
        # adj: sorted kc cols, sorted-exercise rows, transpose, chunk
        As = np.zeros((M, NKC), dtype=f)
        real = order < adj.shape[1]
        As[:ROWS, real] = np.asarray(adj[sl], dtype=f)[perm][:, order[real]]
        At = As.T.reshape(KCH, P, M)                           # [kk, p, i]
        m_c = {"rowB": rowB_c, "exTb": exTb_c, **shared}
        a4b = np.zeros((P, NBF * M), dtype=NP_BF16)
        for kk in range(NBF):
            a4b[:, kk * M:(kk + 1) * M] = At[kk]
        m_c["adjTb"] = a4b
        for g in range(3):
            ag = np.zeros((P, 4 * M), dtype=NP_FP8)
            for o in range(4):
                ag[:, o * M:(o + 1) * M] = At[NBF + g * 4 + o]
            m_c[f"adjg{g}"] = ag
        maps.append(m_c)
    return maps, np.asarray(rd_b, dtype=np.float64), tuple(int(t) for t in Ts), perms


def kernel(exercise_h, kc_h, adj, W1, E, a, rd_w, rd_b):
    maps, rdb, Ts, perms = _prep(exercise_h, kc_h, adj, W1, E, a, rd_w, rd_b)
    nc = _get_program(Ts)
    res = run_bass_kernel_spmd(nc, maps, list(range(NCORES))).results
    out = np.empty((N_E, D), dtype=np.float32)
    for cidx in range(NCORES):
        outBv = np.asarray(res[cidx]["outB"]).astype(np.float64)
        s = np.asarray(res[cidx]["srow"]).astype(np.float64)[0, :ROWS]
        A = outBv.reshape(P, M, 2)
        raw = np.concatenate([A[:, :ROWS, 0].T, A[:, :ROWS, 1].T], axis=1)
        o = raw / s[:, None] + rdb[None, :]
        o = np.where(o > 0, o, np.expm1(np.minimum(o, 0)))
        inv = np.empty(ROWS, dtype=np.int64)
        inv[perms[cidx]] = np.arange(ROWS)
        out[cidx * ROWS:(cidx + 1) * ROWS] = o[inv].astype(np.float32)
    return out
